# revision 25
# baseline (speedup 1.0000x reference)
"""Trainium2 Bass kernel for the DriftingLoss nn module.

Reference math (x, y_pos, y_neg all [4096, 256] fp32):
  scale^2 = mean(concat(y_pos, y_neg)^2) + 1e-8
  c_ij    = (||y_j||^2 - 2 x_i . y_j) / scale^2     (row-constant ||x_i||^2
                                                     dropped: it cancels in the
                                                     per-row softmax; the
                                                     max(.,0) clamp in the
                                                     reference never fires for
                                                     this data regime)
  For tau in (0.02, 0.05, 0.2), sign in (pos, neg):
    w = softmax_j(-c_ij / tau);  U_ts = w @ y_s     (y unscaled: equals the
                                                     reference's scaled-y field
                                                     times scale)
  D_t = U_t_pos - U_t_neg;  V_raw = mean_t D_t
  raw = mean_i ||V_raw_i||^2;  lam = sqrt(raw/256 + 1e-8);  V = V_raw / lam
  loss = mean(V*V);  drift_norm = mean_i ||V_i||^2;  per_temp_t = mean ||D_t||^2

Sharding: data-parallel over rows of x (512 rows/core on 8 cores), y_pos/y_neg
replicated; the 4 scalar statistics are all-reduced on device (a dummy warmup
collective issued at kernel start absorbs the ~75us first-collective latency,
leaving ~17us for the real one).

Numerics notes:
 - pass 1 (row stabilizers mn_i) runs on UNSCALED centered values so it does
   not serialize behind the scale^2 reduction; mn only stabilizes the exp and
   any per-row error cancels between numerator and denominator of the softmax.
 - values are centered by BB_CENTER (~E||y_j||^2) so bf16 rounding of the mn
   row stays ~0.7 absolute -> exp args stay well inside fp32 range.
 - the distance matmul (pass 2) must be fp32: an absolute error eps in
   x.y shifts exp args by 2*eps/tau (tau=0.02 amplifies 100x).
 - the U = w @ y matmul runs in float32r (tf32-like, 4x faster than fp32):
   e in [0,1] and y O(1), so the ~5e-4 relative rounding lands well inside
   the fp32 softmax envelope.
 - PSUM: matmul start=True clears has_written bits for the WHOLE bank, so the
   two half-bank U accumulators sharing a bank are initialized by one K=1 zero
   matmul per bank and all real U matmuls accumulate with start=False.
"""

import os
import sys
import types

import numpy as np

N, D = 4096, 256
N_CORES = 8
I_LOC = N // N_CORES          # 512 rows per core
N_IT = I_LOC // 128           # 4 i-tiles
N_JT = N // 128               # 32 j-tiles (pass 2 / U)
N_JC = N // 512               # 8 512-wide j-chunks (pass 1)
TAUS = (0.02, 0.05, 0.2)
NT = len(TAUS)
EPS = 1e-8
BB_CENTER = 256.0

_CACHE = {}


def _install_ntff_hook():
    """Fabricate antenv.axon_hooks so run_bass_kernel_spmd(trace=True) can
    profile through the axon PJRT .so (the agent image ships without it)."""
    try:
        import antenv

        if "antenv.axon_hooks" in sys.modules:
            return
        mod = types.ModuleType("antenv.axon_hooks")
        _h = {"hook": None}
        mod.set_axon_ntff_profile_hook = lambda h: _h.__setitem__("hook", h)
        mod.get_axon_ntff_profile_hook = lambda: _h["hook"]
        sys.modules["antenv.axon_hooks"] = mod
        antenv.axon_hooks = mod
        from trn_agent_boot.trn_boot import _ntff_profile_via_ctypes

        so = "/opt/axon/libaxon_pjrt.so"
        if os.path.exists(so):
            mod.set_axon_ntff_profile_hook(_ntff_profile_via_ctypes(so))
    except Exception:
        pass


def build():
    from contextlib import ExitStack

    from concourse import bacc, bass_isa, masks, mybir, tile

    f32 = mybir.dt.float32
    f32r = mybir.dt.float32r
    bf16 = mybir.dt.bfloat16
    AL = mybir.AluOpType
    AF = mybir.ActivationFunctionType
    AX = mybir.AxisListType

    nc = bacc.Bacc("TRN2", target_bir_lowering=False, debug=False,
                   num_devices=N_CORES)

    x_d = nc.dram_tensor("x", [I_LOC, D], f32, kind="ExternalInput").ap()
    y_d = {
        "p": nc.dram_tensor("y_pos", [N, D], f32, kind="ExternalInput").ap(),
        "n": nc.dram_tensor("y_neg", [N, D], f32, kind="ExternalInput").ap(),
    }
    v_d = nc.dram_tensor("V", [I_LOC, D], f32, kind="ExternalOutput").ap()
    st_d = nc.dram_tensor("stats", [1, 8], f32, kind="ExternalOutput").ap()

    def _trace(tc, ctx):
        singles = ctx.enter_context(tc.tile_pool(name="singles", bufs=1))
        small = ctx.enter_context(tc.tile_pool(name="small", bufs=2))
        scrp = ctx.enter_context(tc.tile_pool(name="scrp", bufs=2))
        epool = ctx.enter_context(tc.tile_pool(name="epool", bufs=2))
        ybp = ctx.enter_context(tc.tile_pool(name="ybp", bufs=4))
        rowp = ctx.enter_context(tc.tile_pool(name="rowp", bufs=1))
        outp = ctx.enter_context(tc.tile_pool(name="outp", bufs=2))
        sgp = ctx.enter_context(tc.tile_pool(name="sgp", bufs=1))
        dram = ctx.enter_context(tc.tile_pool(name="dram", bufs=1,
                                              space="DRAM"))
        psum_big = ctx.enter_context(
            tc.tile_pool(name="psum_big", bufs=2, space="PSUM"))
        psum_u = ctx.enter_context(
            tc.tile_pool(name="psum_u", bufs=1, space="PSUM"))

        use_dummy = os.environ.get("KDUMMYCC", "1") == "1"

        # ------------- loads -------------
        x_nat = singles.tile([128, N_IT, D], f32)
        nc.sync.dma_start(out=x_nat,
                          in_=x_d.rearrange("(t p) d -> p t d", p=128))
        y_nat = {}
        for s in ("p", "n"):
            y_nat[s] = ybp.tile([128, N_JT, D], f32, tag="yb",
                                name=f"ynat{s}")
            yr_ = y_d[s].rearrange("(t p) d -> p t d", p=128)
            for k in range(4):
                ksl = slice(k * 8, (k + 1) * 8)
                nc.sync.dma_start(out=y_nat[s][:, ksl], in_=yr_[:, ksl])

        ident = singles.tile([128, 128], f32)
        masks.make_identity(nc, ident)
        ones_row = singles.tile([1, 128], bf16)
        nc.vector.memset(ones_row, 1.0)
        zrow_l = singles.tile([1, 128], bf16)
        nc.vector.memset(zrow_l, 0.0)
        zrow_r = singles.tile([1, 512], bf16)
        nc.vector.memset(zrow_r, 0.0)

        # --- x transposes (unscaled): xTU fp32, xT1 bf16 ---
        xTU = singles.tile([128, 2, I_LOC], f32)
        xT1 = singles.tile([128, 2, I_LOC], bf16)
        for it in range(N_IT):
            for dc in range(2):
                pt = psum_big.tile([128, 512], f32, tag="big")
                nc.tensor.transpose(
                    pt[:, 0:128], x_nat[:, it, dc * 128:(dc + 1) * 128],
                    ident)
                dst = xTU[:, dc, it * 128:(it + 1) * 128]
                # xTU carries the factor 2 of -2 x.y (unscaled space)
                nc.vector.tensor_scalar_mul(dst, pt[:, 0:128], 2.0)
                nc.vector.tensor_copy(
                    out=xT1[:, dc, it * 128:(it + 1) * 128], in_=dst)

        # --- phased prep/pass1. ybp slot budget is 4; order matters:
        #   ynat_p, ynat_n, yT_p, yT1_p | y_r_p | yT1_n | (post pass2-p)
        #   yT_n, y_r_n
        bb = {}
        yT = {}
        yT1 = {}
        nbbrow = {}
        y_r = {}
        mnT = {}

        def prep_bb_row(s):
            bb[s] = singles.tile([128, N_JT], f32, tag=f"bb{s}",
                                 name=f"bb{s}")
            for jt in range(N_JT):
                scr = scrp.tile([128, D], f32, tag="bbscr")
                nc.scalar.activation(
                    out=scr, in_=y_nat[s][:, jt, :], func=AF.Square,
                    accum_out=bb[s][:, jt:jt + 1])
            nb = small.tile([128, N_JT], f32, tag="nbb")
            nc.vector.tensor_scalar(out=nb, in0=bb[s], scalar1=BB_CENTER,
                                    scalar2=-1.0, op0=AL.subtract,
                                    op1=AL.mult)
            ptn = psum_big.tile([128, 512], f32, tag="big")
            nc.tensor.transpose(ptn[0:N_JT, 0:128], nb, ident)
            nbT = small.tile([N_JT, 128], bf16, tag="nbT")
            nc.vector.tensor_copy(out=nbT, in_=ptn[0:N_JT, 0:128])
            nbbrow[s] = rowp.tile([1, N], bf16, tag="nbbrow",
                                  name=f"nbbrow{s}")
            nc.sync.dma_start(
                out=nbbrow[s].rearrange("o (jt p) -> o jt p", p=128),
                in_=nbT)

        def build_yT(s, fp32_copy, bf16_copy):
            if fp32_copy:
                yT[s] = ybp.tile([128, 2, N], f32, tag="yb", name=f"yT{s}")
            if bf16_copy:
                yT1[s] = ybp.tile([128, 2, N], bf16, tag="yb",
                                  name=f"yT1{s}")
            for jt in range(N_JT):
                for dc in range(2):
                    pt = psum_big.tile([128, 512], f32, tag="big")
                    nc.tensor.transpose(
                        pt[:, 0:128],
                        y_nat[s][:, jt, dc * 128:(dc + 1) * 128], ident)
                    if fp32_copy:
                        nc.vector.tensor_copy(
                            out=yT[s][:, dc, jt * 128:(jt + 1) * 128],
                            in_=pt[:, 0:128])
                    if bf16_copy:
                        src_ap = (yT[s][:, dc, jt * 128:(jt + 1) * 128]
                                  if fp32_copy else pt[:, 0:128])
                        nc.vector.tensor_copy(
                            out=yT1[s][:, dc, jt * 128:(jt + 1) * 128],
                            in_=src_ap)

        def pass1(s):
            negmn_parts = small.tile([128, N_IT, N_JC], f32, tag="negmnp",
                                     name=f"negmnp{s}")
            for it in range(N_IT):
                isl = slice(it * 128, (it + 1) * 128)
                for jc in range(N_JC):
                    jsl = slice(jc * 512, (jc + 1) * 512)
                    p1 = psum_big.tile([128, 512], f32, tag="big")
                    nc.tensor.matmul(p1, xT1[:, 0, isl], yT1[s][:, 0, jsl],
                                     start=True, stop=False)
                    nc.tensor.matmul(p1, xT1[:, 1, isl], yT1[s][:, 1, jsl],
                                     start=False, stop=False)
                    nc.tensor.matmul(p1, ones_row, nbbrow[s][:, jsl],
                                     start=False, stop=True)
                    nc.vector.tensor_reduce(
                        negmn_parts[:, it, jc:jc + 1], p1, axis=AX.X,
                        op=AL.max)
            mnT[s] = small.tile([128, N_IT], f32, tag="mnT", name=f"mnT{s}")
            nc.vector.tensor_reduce(mnT[s], negmn_parts, axis=AX.X, op=AL.max)

        prep_bb_row("p")
        build_yT("p", fp32_copy=True, bf16_copy=True)
        pass1("p")
        y_r["p"] = ybp.tile([128, N_JT, D], f32r, tag="yb", name="yrp")
        nc.vector.tensor_copy(out=y_r["p"], in_=y_nat["p"])
        prep_bb_row("n")
        build_yT("n", fp32_copy=False, bf16_copy=True)
        pass1("n")

        # --- scale chain (needs both bb's); partition-reduce on PE so it
        # is not queued behind the warmup collective on gpsimd ---
        tot = small.tile([128, 1], f32, tag="tot")
        tot2 = small.tile([128, 1], f32, tag="tot2")
        nc.vector.tensor_reduce(tot, bb["p"], axis=AX.X, op=AL.add)
        nc.vector.tensor_reduce(tot2, bb["n"], axis=AX.X, op=AL.add)
        nc.vector.tensor_add(tot, tot, tot2)
        ones_col = singles.tile([128, 1], f32)
        nc.vector.memset(ones_col, 1.0)
        ptot = psum_big.tile([128, 512], f32, tag="big")
        nc.tensor.matmul(ptot[0:1, 0:1], tot, ones_col, start=True, stop=True)
        tot1 = small.tile([1, 1], f32, tag="tot1")
        nc.vector.tensor_scalar(out=tot1, in0=ptot[0:1, 0:1],
                                scalar1=1.0 / (2 * N * D), scalar2=EPS,
                                op0=AL.mult, op1=AL.add)
        totd = dram.tile([1, 1], f32, tag="totd")
        nc.sync.dma_start(out=totd, in_=tot1)
        scale2 = singles.tile([128, 1], f32)
        nc.sync.dma_start(out=scale2, in_=totd.to_broadcast([128, 1]))
        rs2 = singles.tile([128, 1], f32)
        nc.vector.reciprocal(rs2, scale2)
        rt = small.tile([128, 1], f32, tag="rt")
        nc.vector.tensor_mul(rt, scale2, rs2)
        nc.vector.tensor_scalar(out=rt, in0=rt, scalar1=-1.0, scalar2=2.0,
                                op0=AL.mult, op1=AL.add)
        nc.vector.tensor_mul(rs2, rs2, rt)          # Newton: rs2*(2-scale2*rs2)
        # xT2 = xTU / scale^2   (xTU already carries the factor 2)
        xT2 = singles.tile([128, 2, I_LOC], f32)
        nc.vector.tensor_scalar_mul(xT2, xTU, rs2)

        # negbb[s][t] = (BB_CENTER - bb)/scale^2/tau ; mn rows (scaled, bf16)
        negbb = {}
        mn_row = {}
        for s in ("p", "n"):
            bbsK = small.tile([128, N_JT], f32, tag="bbsK", name=f"bbsK{s}")
            nc.vector.tensor_scalar(out=bbsK, in0=bb[s], scalar1=BB_CENTER,
                                    scalar2=rs2, op0=AL.subtract, op1=AL.mult)
            negbb[s] = singles.tile([128, NT, N_JT], f32, tag=f"negbb{s}",
                                    name=f"negbb{s}")
            for t, tau in enumerate(TAUS):
                nc.vector.tensor_scalar_mul(negbb[s][:, t, :], bbsK,
                                            -1.0 / tau)
            mnTb = small.tile([128, N_IT], f32, tag="mnTb")
            negrs2 = small.tile([128, 1], f32, tag="negrs2")
            nc.vector.tensor_scalar_mul(negrs2, rs2, -1.0)
            nc.vector.tensor_scalar_mul(mnTb, mnT[s], negrs2)
            ptm = psum_big.tile([128, 512], f32, tag="big")
            nc.tensor.transpose(ptm[0:N_IT, 0:128], mnTb, ident)
            mnTT = small.tile([N_IT, 128], bf16, tag="mnTT")
            nc.vector.tensor_copy(out=mnTT, in_=ptm[0:N_IT, 0:128])
            mn_row[s] = small.tile([1, I_LOC], bf16, tag="mn_row",
                                   name=f"mn_row{s}")
            nc.sync.dma_start(
                out=mn_row[s].rearrange("o (t p) -> o t p", p=128),
                in_=mnTT)

        # U flush tiles (pos results; overwritten with -D after neg)
        P = {}
        for t in range(NT):
            for it in range(N_IT):
                P[(t, it)] = singles.tile([128, D], f32, tag=f"P{t}_{it}",
                                          name=f"P{t}_{it}")
        statc = singles.tile([128, 4, N_IT], f32)
        rsP = {}

        # ---- dummy collective: absorb comm warmup; overlaps pass2-p ----
        zz = singles.tile([1, 8], f32)
        nc.vector.memset(zz, 0.0)
        if use_dummy:
            cc_din = dram.tile([1, 8], f32, tag="cc_din")
            cc_dout = dram.tile([1, 8], f32, tag="cc_dout")
            nc.gpsimd.dma_start(out=cc_din, in_=zz)
            nc.gpsimd.collective_compute(
                "AllReduce", AL.add, replica_groups=[list(range(N_CORES))],
                ins=[cc_din.opt()], outs=[cc_dout.opt()])
            warm = singles.tile([1, 8], f32)
            nc.sync.dma_start(out=warm, in_=cc_dout)
        else:
            warm = zz

        # =================== per-sign main phases ===================
        for s in ("p", "n"):
            if s == "n":
                # late builds into slots freed by pass2-p
                build_yT("n", fp32_copy=True, bf16_copy=False)
                y_r["n"] = ybp.tile([128, N_JT, D], f32r, tag="yb",
                                    name="yrn")
                nc.vector.tensor_copy(out=y_r["n"], in_=y_nat["n"])

            s_acc = [small.tile([128, I_LOC], f32, tag=f"s_acc{t}",
                                name=f"s_acc{t}") for t in range(NT)]
            for t in range(NT):
                nc.vector.memset(s_acc[t], 0.0)
            ubank = [psum_u.tile([128, 512], f32, tag=f"U{t}_{h}",
                                 name=f"U{t}_{h}") for t in range(NT)
                     for h in range(2)]
            for ub in ubank:
                nc.tensor.matmul(ub, zrow_l, zrow_r, start=True, stop=False,
                                 skip_group_check=True)

            for jt in range(N_JT):
                jsl = slice(jt * 128, (jt + 1) * 128)
                p2 = psum_big.tile([128, 512], f32, tag="big")
                nc.tensor.matmul(p2, yT[s][:, 0, jsl], xT2[:, 0, :],
                                 start=True, stop=False)
                nc.tensor.matmul(p2, yT[s][:, 1, jsl], xT2[:, 1, :],
                                 start=False, stop=False)
                nc.tensor.matmul(p2, ones_row, mn_row[s],
                                 start=False, stop=True)
                for t, tau in enumerate(TAUS):
                    e = epool.tile([128, I_LOC], f32r, tag=f"e{t}")
                    nc.scalar.activation(
                        out=e, in_=p2, func=AF.Exp,
                        bias=negbb[s][:, t, jt:jt + 1], scale=1.0 / tau)
                    nc.vector.tensor_add(s_acc[t], s_acc[t], e)
                    for it in range(N_IT):
                        half = it % 2
                        ub = ubank[t * 2 + it // 2]
                        nc.tensor.matmul(
                            ub[:, half * D:(half + 1) * D],
                            e[:, it * 128:(it + 1) * 128],
                            y_r[s][:, jt, :],
                            start=False, stop=(jt == N_JT - 1),
                            skip_group_check=True)

            # --- s partition-sum -> 1/s as [128, N_IT] columns ---
            rsP[s] = small.tile([128, NT, N_IT], f32, tag="rsP",
                                name=f"rsP{s}")
            for t in range(NT):
                psg = psum_big.tile([128, 512], f32, tag="big")
                nc.tensor.matmul(psg[0:1, :], ones_col, s_acc[t],
                                 start=True, stop=True)
                sg = sgp.tile([128, I_LOC], f32, tag="sg")
                nc.vector.tensor_copy(out=sg[0:1, :], in_=psg[0:1, :])
                pts = psum_big.tile([128, 512], f32, tag="big")
                for it in range(N_IT):
                    nc.tensor.transpose(
                        pts[:, it:it + 1],
                        sg[0:1, it * 128:(it + 1) * 128], ident[0:1, 0:1])
                sv = small.tile([128, N_IT], f32, tag="sv")
                nc.vector.tensor_copy(out=sv, in_=pts[:, 0:N_IT])
                dst = rsP[s][:, t, :]
                nc.vector.reciprocal(dst, sv)
                # one Newton step: r*(2 - s*r)
                svt = small.tile([128, N_IT], f32, tag="svt")
                nc.vector.tensor_mul(svt, sv, dst)
                nc.vector.tensor_scalar(out=svt, in0=svt, scalar1=-1.0,
                                        scalar2=2.0, op0=AL.mult, op1=AL.add)
                nc.vector.tensor_mul(dst, dst, svt)

            # --- combine U into P (pos) or -D (neg) ---
            for t in range(NT):
                for it in range(N_IT):
                    half = it % 2
                    ub = ubank[t * 2 + it // 2][:, half * D:(half + 1) * D]
                    if s == "p":
                        nc.vector.tensor_scalar_mul(
                            P[(t, it)], ub, rsP["p"][:, t, it:it + 1])
                    else:
                        # P := U_n/s_n - P  (= -D_t)
                        nc.vector.scalar_tensor_tensor(
                            out=P[(t, it)], in0=ub,
                            scalar=rsP["n"][:, t, it:it + 1],
                            in1=P[(t, it)], op0=AL.mult, op1=AL.subtract)

        # =================== epilogue ===================
        W = []
        for it in range(N_IT):
            for t in range(NT):
                scr = scrp.tile([128, D], f32, tag="stscr")
                nc.scalar.activation(
                    out=scr, in_=P[(t, it)], func=AF.Square,
                    accum_out=statc[:, t, it:it + 1])
            w = singles.tile([128, D], f32, tag=f"wsum{it}", name=f"w{it}")
            nc.vector.tensor_add(w, P[(0, it)], P[(1, it)])
            nc.vector.tensor_add(w, w, P[(2, it)])
            W.append(w)
            scr = scrp.tile([128, D], f32, tag="stscr")
            nc.scalar.activation(
                out=scr, in_=w, func=AF.Square,
                accum_out=statc[:, 3, it:it + 1])
        statr = small.tile([128, 4], f32, tag="statr")
        nc.vector.tensor_reduce(statr, statc, axis=AX.X, op=AL.add)
        pstat = psum_big.tile([128, 512], f32, tag="big")
        nc.tensor.matmul(pstat[0:1, 0:4], ones_col, statr,
                         start=True, stop=True)

        # --- all-reduce the 4 partials across cores ---
        cc_in = small.tile([1, 8], f32, tag="cc_in")
        nc.vector.memset(cc_in, 0.0)
        nc.vector.tensor_copy(out=cc_in[0:1, 0:4], in_=pstat[0:1, 0:4])
        nc.vector.scalar_tensor_tensor(
            out=cc_in, in0=warm, scalar=0.0, in1=cc_in,
            op0=AL.mult, op1=AL.add)
        cc_in_d = dram.tile([1, 8], f32, tag="cc_in_d")
        cc_out_d = dram.tile([1, 8], f32, tag="cc_out_d")
        nc.gpsimd.dma_start(out=cc_in_d, in_=cc_in)
        nc.gpsimd.collective_compute(
            "AllReduce", AL.add, replica_groups=[list(range(N_CORES))],
            ins=[cc_in_d.opt()], outs=[cc_out_d.opt()])
        gl = small.tile([128, 8], f32, tag="gl")
        nc.sync.dma_start(out=gl, in_=cc_out_d.to_broadcast([128, 8]))

        # --- scalar chain (all lanes compute the same values) ---
        def newton_sqrt(dst_tag, a):
            y0 = small.tile([128, 1], f32, tag=dst_tag + "y0")
            nc.scalar.activation(out=y0, in_=a, func=AF.Sqrt)
            r0 = small.tile([128, 1], f32, tag=dst_tag + "r0")
            nc.vector.reciprocal(r0, y0)
            t1 = small.tile([128, 1], f32, tag=dst_tag + "t1")
            nc.vector.tensor_mul(t1, a, r0)
            nc.vector.tensor_add(t1, t1, y0)
            y1 = small.tile([128, 1], f32, tag=dst_tag + "y1")
            nc.vector.tensor_scalar_mul(y1, t1, 0.5)
            return y1

        def newton_recip(dst_tag, a):
            r0 = small.tile([128, 1], f32, tag=dst_tag + "r0")
            nc.vector.reciprocal(r0, a)
            t1 = small.tile([128, 1], f32, tag=dst_tag + "t1")
            nc.vector.tensor_mul(t1, a, r0)
            nc.vector.tensor_scalar(out=t1, in0=t1, scalar1=-1.0, scalar2=2.0,
                                    op0=AL.mult, op1=AL.add)
            nc.vector.tensor_mul(r0, r0, t1)
            return r0

        raw = small.tile([128, 1], f32, tag="raw")
        nc.vector.tensor_scalar_mul(raw, gl[:, 3:4], 1.0 / (9.0 * N))
        lam2 = small.tile([128, 1], f32, tag="lam2")
        nc.vector.tensor_scalar(out=lam2, in0=raw, scalar1=1.0 / D,
                                scalar2=EPS, op0=AL.mult, op1=AL.add)
        lam = newton_sqrt("lam", lam2)
        rlam = newton_recip("rlam", lam)
        rlam2 = small.tile([128, 1], f32, tag="rlam2")
        nc.vector.tensor_mul(rlam2, rlam, rlam)
        drift = small.tile([128, 1], f32, tag="drift")
        nc.vector.tensor_mul(drift, raw, rlam2)
        loss = small.tile([128, 1], f32, tag="loss")
        nc.vector.tensor_scalar_mul(loss, drift, 1.0 / D)
        negf3 = small.tile([128, 1], f32, tag="negf3")
        nc.vector.tensor_scalar_mul(negf3, rlam, -1.0 / 3.0)

        st = small.tile([1, 8], f32, tag="st")
        nc.vector.memset(st, 0.0)
        nc.vector.tensor_copy(out=st[0:1, 0:1], in_=loss[0:1, :])
        nc.vector.tensor_copy(out=st[0:1, 1:2], in_=drift[0:1, :])
        nc.vector.tensor_copy(out=st[0:1, 2:3], in_=raw[0:1, :])
        nc.vector.tensor_copy(out=st[0:1, 3:4], in_=lam[0:1, :])
        for t in range(NT):
            nc.vector.tensor_scalar_mul(st[0:1, 4 + t:5 + t],
                                        gl[0:1, t:t + 1], 1.0 / N)
        nc.sync.dma_start(out=st_d, in_=st)

        # --- V out ---
        for it in range(N_IT):
            vo = outp.tile([128, D], f32, tag="vo")
            nc.vector.tensor_scalar_mul(vo, W[it], negf3)
            nc.sync.dma_start(out=v_d[it * 128:(it + 1) * 128, :], in_=vo)

    with tile.TileContext(nc) as tc, ExitStack() as ctx:
        _trace(tc, ctx)
    nc.compile()
    return nc


def _get_nc():
    if "nc" not in _CACHE:
        _install_ntff_hook()
        _CACHE["nc"] = build()
    return _CACHE["nc"]


def run(inputs, trace=False):
    from concourse import bass_utils

    nc = _get_nc()
    x = np.ascontiguousarray(inputs["x"], dtype=np.float32)
    yp = np.ascontiguousarray(inputs["y_pos"], dtype=np.float32)
    yn = np.ascontiguousarray(inputs["y_neg"], dtype=np.float32)
    in_maps = [
        {"x": x[c * I_LOC:(c + 1) * I_LOC], "y_pos": yp, "y_neg": yn}
        for c in range(N_CORES)
    ]
    res = bass_utils.run_bass_kernel_spmd(
        nc, in_maps, core_ids=list(range(N_CORES)), trace=trace)
    V = np.concatenate([res.results[c]["V"] for c in range(N_CORES)], axis=0)
    st = res.results[0]["stats"][0]
    out = (
        np.float32(st[0]),            # loss
        np.float32(st[1]),            # drift_norm
        np.float32(st[2]),            # raw_drift_norm
        np.float32(st[3]),            # lambda_V
        st[4:7].astype(np.float32),   # per_temp
        V,                            # V [4096, 256]
    )
    return out, res


def kernel(**inputs):
    out, _ = run(inputs, trace=False)
    return out


# revision 26
# speedup vs baseline: 1.0081x; 1.0081x over previous
"""Trainium2 Bass kernel for the DriftingLoss nn module.

Reference math (x, y_pos, y_neg all [4096, 256] fp32):
  scale^2 = mean(concat(y_pos, y_neg)^2) + 1e-8
  c_ij    = (||y_j||^2 - 2 x_i . y_j) / scale^2     (row-constant ||x_i||^2
                                                     dropped: it cancels in the
                                                     per-row softmax; the
                                                     max(.,0) clamp in the
                                                     reference never fires for
                                                     this data regime)
  For tau in (0.02, 0.05, 0.2), sign in (pos, neg):
    w = softmax_j(-c_ij / tau);  U_ts = w @ y_s     (y unscaled: equals the
                                                     reference's scaled-y field
                                                     times scale)
  D_t = U_t_pos - U_t_neg;  V_raw = mean_t D_t
  raw = mean_i ||V_raw_i||^2;  lam = sqrt(raw/256 + 1e-8);  V = V_raw / lam
  loss = mean(V*V);  drift_norm = mean_i ||V_i||^2;  per_temp_t = mean ||D_t||^2

Sharding: data-parallel over rows of x (512 rows/core on 8 cores), y_pos/y_neg
replicated; the 4 scalar statistics are all-reduced on device (a dummy warmup
collective issued at kernel start absorbs the ~75us first-collective latency,
leaving ~17us for the real one).

Numerics notes:
 - pass 1 (row stabilizers mn_i) runs on UNSCALED centered values so it does
   not serialize behind the scale^2 reduction; mn only stabilizes the exp and
   any per-row error cancels between numerator and denominator of the softmax.
 - values are centered by BB_CENTER (~E||y_j||^2) so bf16 rounding of the mn
   row stays ~0.7 absolute -> exp args stay well inside fp32 range.
 - the distance matmul (pass 2) must be fp32: an absolute error eps in
   x.y shifts exp args by 2*eps/tau (tau=0.02 amplifies 100x).
 - the U = w @ y matmul runs in float32r (tf32-like, 4x faster than fp32):
   e in [0,1] and y O(1), so the ~5e-4 relative rounding lands well inside
   the fp32 softmax envelope.
 - PSUM: matmul start=True clears has_written bits for the WHOLE bank, so the
   two half-bank U accumulators sharing a bank are initialized by one K=1 zero
   matmul per bank and all real U matmuls accumulate with start=False.
"""

import os
import sys
import types

import numpy as np

N, D = 4096, 256
N_CORES = 8
I_LOC = N // N_CORES          # 512 rows per core
N_IT = I_LOC // 128           # 4 i-tiles
N_JT = N // 128               # 32 j-tiles (pass 2 / U)
N_JC = N // 512               # 8 512-wide j-chunks (pass 1)
TAUS = (0.02, 0.05, 0.2)
NT = len(TAUS)
EPS = 1e-8
BB_CENTER = 256.0

_CACHE = {}


def _install_ntff_hook():
    """Fabricate antenv.axon_hooks so run_bass_kernel_spmd(trace=True) can
    profile through the axon PJRT .so (the agent image ships without it)."""
    try:
        import antenv

        if "antenv.axon_hooks" in sys.modules:
            return
        mod = types.ModuleType("antenv.axon_hooks")
        _h = {"hook": None}
        mod.set_axon_ntff_profile_hook = lambda h: _h.__setitem__("hook", h)
        mod.get_axon_ntff_profile_hook = lambda: _h["hook"]
        sys.modules["antenv.axon_hooks"] = mod
        antenv.axon_hooks = mod
        from trn_agent_boot.trn_boot import _ntff_profile_via_ctypes

        so = "/opt/axon/libaxon_pjrt.so"
        if os.path.exists(so):
            mod.set_axon_ntff_profile_hook(_ntff_profile_via_ctypes(so))
    except Exception:
        pass


def build():
    from contextlib import ExitStack

    from concourse import bacc, bass_isa, masks, mybir, tile

    f32 = mybir.dt.float32
    f32r = mybir.dt.float32r
    bf16 = mybir.dt.bfloat16
    AL = mybir.AluOpType
    AF = mybir.ActivationFunctionType
    AX = mybir.AxisListType

    nc = bacc.Bacc("TRN2", target_bir_lowering=False, debug=False,
                   num_devices=N_CORES)

    x_d = nc.dram_tensor("x", [I_LOC, D], f32, kind="ExternalInput").ap()
    y_d = {
        "p": nc.dram_tensor("y_pos", [N, D], f32, kind="ExternalInput").ap(),
        "n": nc.dram_tensor("y_neg", [N, D], f32, kind="ExternalInput").ap(),
    }
    v_d = nc.dram_tensor("V", [I_LOC, D], f32, kind="ExternalOutput").ap()
    st_d = nc.dram_tensor("stats", [1, 8], f32, kind="ExternalOutput").ap()

    def _trace(tc, ctx):
        singles = ctx.enter_context(tc.tile_pool(name="singles", bufs=1))
        small = ctx.enter_context(tc.tile_pool(name="small", bufs=2))
        scrp = ctx.enter_context(tc.tile_pool(name="scrp", bufs=2))
        epool = ctx.enter_context(tc.tile_pool(name="epool", bufs=2))
        ybp = ctx.enter_context(tc.tile_pool(name="ybp", bufs=4))
        rowp = ctx.enter_context(tc.tile_pool(name="rowp", bufs=1))
        outp = ctx.enter_context(tc.tile_pool(name="outp", bufs=2))
        sgp = ctx.enter_context(tc.tile_pool(name="sgp", bufs=1))
        dram = ctx.enter_context(tc.tile_pool(name="dram", bufs=1,
                                              space="DRAM"))
        psum_big = ctx.enter_context(
            tc.tile_pool(name="psum_big", bufs=2, space="PSUM"))
        psum_u = ctx.enter_context(
            tc.tile_pool(name="psum_u", bufs=1, space="PSUM"))

        use_dummy = os.environ.get("KDUMMYCC", "1") == "1"

        # ------------- loads -------------
        x_nat = singles.tile([128, N_IT, D], f32)
        nc.sync.dma_start(out=x_nat,
                          in_=x_d.rearrange("(t p) d -> p t d", p=128))
        y_nat = {}
        for s in ("p", "n"):
            y_nat[s] = ybp.tile([128, N_JT, D], f32, tag="yb",
                                name=f"ynat{s}")
            yr_ = y_d[s].rearrange("(t p) d -> p t d", p=128)
            for k in range(4):
                ksl = slice(k * 8, (k + 1) * 8)
                nc.sync.dma_start(out=y_nat[s][:, ksl], in_=yr_[:, ksl])

        ident = singles.tile([128, 128], f32)
        masks.make_identity(nc, ident)
        ones_row = singles.tile([1, 128], bf16)
        nc.vector.memset(ones_row, 1.0)
        zrow_l = singles.tile([1, 128], bf16)
        nc.vector.memset(zrow_l, 0.0)
        zrow_r = singles.tile([1, 512], bf16)
        nc.vector.memset(zrow_r, 0.0)

        # --- x transposes (unscaled): xTU fp32, xT1 bf16 ---
        xTU = singles.tile([128, 2, I_LOC], f32)
        xT1 = singles.tile([128, 2, I_LOC], bf16)
        for it in range(N_IT):
            for dc in range(2):
                pt = psum_big.tile([128, 512], f32, tag="big")
                nc.tensor.transpose(
                    pt[:, 0:128], x_nat[:, it, dc * 128:(dc + 1) * 128],
                    ident)
                dst = xTU[:, dc, it * 128:(it + 1) * 128]
                # xTU carries the factor 2 of -2 x.y (unscaled space)
                nc.vector.tensor_scalar_mul(dst, pt[:, 0:128], 2.0)
                nc.vector.tensor_copy(
                    out=xT1[:, dc, it * 128:(it + 1) * 128], in_=dst)

        # --- phased prep/pass1. ybp slot budget is 4; order matters:
        #   ynat_p, ynat_n, yT_p, yT1_p | y_r_p | yT1_n | (post pass2-p)
        #   yT_n, y_r_n
        bb = {}
        yT = {}
        yT1 = {}
        nbbrow = {}
        y_r = {}
        mnT = {}

        def prep_bb_row(s):
            bb[s] = singles.tile([128, N_JT], f32, tag=f"bb{s}",
                                 name=f"bb{s}")
            for jt in range(N_JT):
                scr = scrp.tile([128, D], f32, tag="bbscr")
                nc.scalar.activation(
                    out=scr, in_=y_nat[s][:, jt, :], func=AF.Square,
                    accum_out=bb[s][:, jt:jt + 1])
            nb = small.tile([128, N_JT], f32, tag="nbb")
            nc.vector.tensor_scalar(out=nb, in0=bb[s], scalar1=BB_CENTER,
                                    scalar2=-1.0, op0=AL.subtract,
                                    op1=AL.mult)
            ptn = psum_big.tile([128, 512], f32, tag="big")
            nc.tensor.transpose(ptn[0:N_JT, 0:128], nb, ident)
            nbT = small.tile([N_JT, 128], bf16, tag="nbT")
            nc.vector.tensor_copy(out=nbT, in_=ptn[0:N_JT, 0:128])
            nbbrow[s] = rowp.tile([1, N], bf16, tag="nbbrow",
                                  name=f"nbbrow{s}")
            nc.sync.dma_start(
                out=nbbrow[s].rearrange("o (jt p) -> o jt p", p=128),
                in_=nbT)

        def build_yT(s, fp32_copy, bf16_copy):
            if fp32_copy:
                yT[s] = ybp.tile([128, 2, N], f32, tag="yb", name=f"yT{s}")
            if bf16_copy:
                yT1[s] = ybp.tile([128, 2, N], bf16, tag="yb",
                                  name=f"yT1{s}")
            for jt in range(N_JT):
                for dc in range(2):
                    pt = psum_big.tile([128, 512], f32, tag="big")
                    nc.tensor.transpose(
                        pt[:, 0:128],
                        y_nat[s][:, jt, dc * 128:(dc + 1) * 128], ident)
                    if fp32_copy:
                        nc.vector.tensor_copy(
                            out=yT[s][:, dc, jt * 128:(jt + 1) * 128],
                            in_=pt[:, 0:128])
                    if bf16_copy:
                        src_ap = (yT[s][:, dc, jt * 128:(jt + 1) * 128]
                                  if fp32_copy else pt[:, 0:128])
                        nc.vector.tensor_copy(
                            out=yT1[s][:, dc, jt * 128:(jt + 1) * 128],
                            in_=src_ap)

        def pass1(s):
            negmn_parts = small.tile([128, N_IT, N_JC], f32, tag="negmnp",
                                     name=f"negmnp{s}")
            for it in range(N_IT):
                isl = slice(it * 128, (it + 1) * 128)
                for jc in range(N_JC):
                    jsl = slice(jc * 512, (jc + 1) * 512)
                    p1 = psum_big.tile([128, 512], f32, tag="big")
                    nc.tensor.matmul(p1, xT1[:, 0, isl], yT1[s][:, 0, jsl],
                                     start=True, stop=False)
                    nc.tensor.matmul(p1, xT1[:, 1, isl], yT1[s][:, 1, jsl],
                                     start=False, stop=False)
                    nc.tensor.matmul(p1, ones_row, nbbrow[s][:, jsl],
                                     start=False, stop=True)
                    nc.vector.tensor_reduce(
                        negmn_parts[:, it, jc:jc + 1], p1, axis=AX.X,
                        op=AL.max)
            mnT[s] = small.tile([128, N_IT], f32, tag="mnT", name=f"mnT{s}")
            nc.vector.tensor_reduce(mnT[s], negmn_parts, axis=AX.X, op=AL.max)

        prep_bb_row("p")
        build_yT("p", fp32_copy=True, bf16_copy=True)
        prep_bb_row("n")     # ACT-only; overlaps pass1-p PE work
        pass1("p")

        # --- scale chain (needs both bb's); partition-reduce on PE so it
        # is not queued behind the warmup collective on gpsimd ---
        tot = small.tile([128, 1], f32, tag="tot")
        tot2 = small.tile([128, 1], f32, tag="tot2")
        nc.vector.tensor_reduce(tot, bb["p"], axis=AX.X, op=AL.add)
        nc.vector.tensor_reduce(tot2, bb["n"], axis=AX.X, op=AL.add)
        nc.vector.tensor_add(tot, tot, tot2)
        ones_col = singles.tile([128, 1], f32)
        nc.vector.memset(ones_col, 1.0)
        ptot = psum_big.tile([128, 512], f32, tag="big")
        nc.tensor.matmul(ptot[0:1, 0:1], tot, ones_col, start=True, stop=True)
        tot1 = small.tile([1, 1], f32, tag="tot1")
        nc.vector.tensor_scalar(out=tot1, in0=ptot[0:1, 0:1],
                                scalar1=1.0 / (2 * N * D), scalar2=EPS,
                                op0=AL.mult, op1=AL.add)
        totd = dram.tile([1, 1], f32, tag="totd")
        nc.sync.dma_start(out=totd, in_=tot1)
        scale2 = singles.tile([128, 1], f32)
        nc.sync.dma_start(out=scale2, in_=totd.to_broadcast([128, 1]))
        rs2 = singles.tile([128, 1], f32)
        nc.vector.reciprocal(rs2, scale2)
        rt = small.tile([128, 1], f32, tag="rt")
        nc.vector.tensor_mul(rt, scale2, rs2)
        nc.vector.tensor_scalar(out=rt, in0=rt, scalar1=-1.0, scalar2=2.0,
                                op0=AL.mult, op1=AL.add)
        nc.vector.tensor_mul(rs2, rs2, rt)          # Newton: rs2*(2-scale2*rs2)
        # xT2 = xTU / scale^2   (xTU already carries the factor 2)
        xT2 = singles.tile([128, 2, I_LOC], f32)
        nc.vector.tensor_scalar_mul(xT2, xTU, rs2)

        # negbb[s][t] = (BB_CENTER - bb)/scale^2/tau ; mn rows (scaled, bf16)
        negbb = {}
        mn_row = {}

        def emit_negbb_mn(s):
            bbsK = small.tile([128, N_JT], f32, tag="bbsK", name=f"bbsK{s}")
            nc.vector.tensor_scalar(out=bbsK, in0=bb[s], scalar1=BB_CENTER,
                                    scalar2=rs2, op0=AL.subtract, op1=AL.mult)
            negbb[s] = singles.tile([128, NT, N_JT], f32, tag=f"negbb{s}",
                                    name=f"negbb{s}")
            for t, tau in enumerate(TAUS):
                nc.vector.tensor_scalar_mul(negbb[s][:, t, :], bbsK,
                                            -1.0 / tau)
            mnTb = small.tile([128, N_IT], f32, tag="mnTb")
            negrs2 = small.tile([128, 1], f32, tag="negrs2")
            nc.vector.tensor_scalar_mul(negrs2, rs2, -1.0)
            nc.vector.tensor_scalar_mul(mnTb, mnT[s], negrs2)
            ptm = psum_big.tile([128, 512], f32, tag="big")
            nc.tensor.transpose(ptm[0:N_IT, 0:128], mnTb, ident)
            mnTT = small.tile([N_IT, 128], bf16, tag="mnTT")
            nc.vector.tensor_copy(out=mnTT, in_=ptm[0:N_IT, 0:128])
            mn_row[s] = small.tile([1, I_LOC], bf16, tag="mn_row",
                                   name=f"mn_row{s}")
            nc.sync.dma_start(
                out=mn_row[s].rearrange("o (t p) -> o t p", p=128),
                in_=mnTT)

        emit_negbb_mn("p")
        y_r["p"] = ybp.tile([128, N_JT, D], f32r, tag="yb", name="yrp")
        nc.vector.tensor_copy(out=y_r["p"], in_=y_nat["p"])
        build_yT("n", fp32_copy=False, bf16_copy=True)
        pass1("n")
        emit_negbb_mn("n")

        # U flush tiles (pos results; overwritten with -D after neg)
        P = {}
        for t in range(NT):
            for it in range(N_IT):
                P[(t, it)] = singles.tile([128, D], f32, tag=f"P{t}_{it}",
                                          name=f"P{t}_{it}")
        statc = singles.tile([128, 4, N_IT], f32)
        rsP = {}

        # ---- dummy collective: absorb comm warmup; overlaps pass2-p ----
        zz = singles.tile([1, 8], f32)
        nc.vector.memset(zz, 0.0)
        if use_dummy:
            cc_din = dram.tile([1, 8], f32, tag="cc_din")
            cc_dout = dram.tile([1, 8], f32, tag="cc_dout")
            nc.gpsimd.dma_start(out=cc_din, in_=zz)
            nc.gpsimd.collective_compute(
                "AllReduce", AL.add, replica_groups=[list(range(N_CORES))],
                ins=[cc_din.opt()], outs=[cc_dout.opt()])
            warm = singles.tile([1, 8], f32)
            nc.sync.dma_start(out=warm, in_=cc_dout)
        else:
            warm = zz

        # =================== per-sign main phases ===================
        for s in ("p", "n"):
            if s == "n":
                # late builds into slots freed by pass2-p
                build_yT("n", fp32_copy=True, bf16_copy=False)
                y_r["n"] = ybp.tile([128, N_JT, D], f32r, tag="yb",
                                    name="yrn")
                nc.vector.tensor_copy(out=y_r["n"], in_=y_nat["n"])

            s_acc = [small.tile([128, I_LOC], f32, tag=f"s_acc{t}",
                                name=f"s_acc{t}") for t in range(NT)]
            for t in range(NT):
                nc.vector.memset(s_acc[t], 0.0)
            ubank = [psum_u.tile([128, 512], f32, tag=f"U{t}_{h}",
                                 name=f"U{t}_{h}") for t in range(NT)
                     for h in range(2)]
            for ub in ubank:
                nc.tensor.matmul(ub, zrow_l, zrow_r, start=True, stop=False,
                                 skip_group_check=True)

            for jt in range(N_JT):
                jsl = slice(jt * 128, (jt + 1) * 128)
                p2 = psum_big.tile([128, 512], f32, tag="big")
                nc.tensor.matmul(p2, yT[s][:, 0, jsl], xT2[:, 0, :],
                                 start=True, stop=False)
                nc.tensor.matmul(p2, yT[s][:, 1, jsl], xT2[:, 1, :],
                                 start=False, stop=False)
                nc.tensor.matmul(p2, ones_row, mn_row[s],
                                 start=False, stop=True)
                for t, tau in enumerate(TAUS):
                    e = epool.tile([128, I_LOC], f32r, tag=f"e{t}")
                    nc.scalar.activation(
                        out=e, in_=p2, func=AF.Exp,
                        bias=negbb[s][:, t, jt:jt + 1], scale=1.0 / tau)
                    nc.vector.tensor_add(s_acc[t], s_acc[t], e)
                    for it in range(N_IT):
                        half = it % 2
                        ub = ubank[t * 2 + it // 2]
                        nc.tensor.matmul(
                            ub[:, half * D:(half + 1) * D],
                            e[:, it * 128:(it + 1) * 128],
                            y_r[s][:, jt, :],
                            start=False, stop=(jt == N_JT - 1),
                            skip_group_check=True)

            # --- s partition-sum -> 1/s as [128, N_IT] columns ---
            rsP[s] = small.tile([128, NT, N_IT], f32, tag="rsP",
                                name=f"rsP{s}")
            for t in range(NT):
                psg = psum_big.tile([128, 512], f32, tag="big")
                nc.tensor.matmul(psg[0:1, :], ones_col, s_acc[t],
                                 start=True, stop=True)
                sg = sgp.tile([128, I_LOC], f32, tag="sg")
                nc.vector.tensor_copy(out=sg[0:1, :], in_=psg[0:1, :])
                pts = psum_big.tile([128, 512], f32, tag="big")
                for it in range(N_IT):
                    nc.tensor.transpose(
                        pts[:, it:it + 1],
                        sg[0:1, it * 128:(it + 1) * 128], ident[0:1, 0:1])
                sv = small.tile([128, N_IT], f32, tag="sv")
                nc.vector.tensor_copy(out=sv, in_=pts[:, 0:N_IT])
                dst = rsP[s][:, t, :]
                nc.vector.reciprocal(dst, sv)
                # one Newton step: r*(2 - s*r)
                svt = small.tile([128, N_IT], f32, tag="svt")
                nc.vector.tensor_mul(svt, sv, dst)
                nc.vector.tensor_scalar(out=svt, in0=svt, scalar1=-1.0,
                                        scalar2=2.0, op0=AL.mult, op1=AL.add)
                nc.vector.tensor_mul(dst, dst, svt)

            # --- combine U into P (pos) or -D (neg) ---
            for t in range(NT):
                for it in range(N_IT):
                    half = it % 2
                    ub = ubank[t * 2 + it // 2][:, half * D:(half + 1) * D]
                    if s == "p":
                        nc.vector.tensor_scalar_mul(
                            P[(t, it)], ub, rsP["p"][:, t, it:it + 1])
                    else:
                        # P := U_n/s_n - P  (= -D_t)
                        nc.vector.scalar_tensor_tensor(
                            out=P[(t, it)], in0=ub,
                            scalar=rsP["n"][:, t, it:it + 1],
                            in1=P[(t, it)], op0=AL.mult, op1=AL.subtract)

        # =================== epilogue ===================
        W = []
        for it in range(N_IT):
            for t in range(NT):
                scr = scrp.tile([128, D], f32, tag="stscr")
                nc.scalar.activation(
                    out=scr, in_=P[(t, it)], func=AF.Square,
                    accum_out=statc[:, t, it:it + 1])
            w = singles.tile([128, D], f32, tag=f"wsum{it}", name=f"w{it}")
            nc.vector.tensor_add(w, P[(0, it)], P[(1, it)])
            nc.vector.tensor_add(w, w, P[(2, it)])
            W.append(w)
            scr = scrp.tile([128, D], f32, tag="stscr")
            nc.scalar.activation(
                out=scr, in_=w, func=AF.Square,
                accum_out=statc[:, 3, it:it + 1])
        statr = small.tile([128, 4], f32, tag="statr")
        nc.vector.tensor_reduce(statr, statc, axis=AX.X, op=AL.add)
        pstat = psum_big.tile([128, 512], f32, tag="big")
        nc.tensor.matmul(pstat[0:1, 0:4], ones_col, statr,
                         start=True, stop=True)

        # --- all-reduce the 4 partials across cores ---
        cc_in = small.tile([1, 8], f32, tag="cc_in")
        nc.vector.memset(cc_in, 0.0)
        nc.vector.tensor_copy(out=cc_in[0:1, 0:4], in_=pstat[0:1, 0:4])
        nc.vector.scalar_tensor_tensor(
            out=cc_in, in0=warm, scalar=0.0, in1=cc_in,
            op0=AL.mult, op1=AL.add)
        cc_in_d = dram.tile([1, 8], f32, tag="cc_in_d")
        cc_out_d = dram.tile([1, 8], f32, tag="cc_out_d")
        nc.gpsimd.dma_start(out=cc_in_d, in_=cc_in)
        nc.gpsimd.collective_compute(
            "AllReduce", AL.add, replica_groups=[list(range(N_CORES))],
            ins=[cc_in_d.opt()], outs=[cc_out_d.opt()])
        gl = small.tile([128, 8], f32, tag="gl")
        nc.sync.dma_start(out=gl, in_=cc_out_d.to_broadcast([128, 8]))

        # --- scalar chain (all lanes compute the same values) ---
        def newton_sqrt(dst_tag, a):
            y0 = small.tile([128, 1], f32, tag=dst_tag + "y0")
            nc.scalar.activation(out=y0, in_=a, func=AF.Sqrt)
            r0 = small.tile([128, 1], f32, tag=dst_tag + "r0")
            nc.vector.reciprocal(r0, y0)
            t1 = small.tile([128, 1], f32, tag=dst_tag + "t1")
            nc.vector.tensor_mul(t1, a, r0)
            nc.vector.tensor_add(t1, t1, y0)
            y1 = small.tile([128, 1], f32, tag=dst_tag + "y1")
            nc.vector.tensor_scalar_mul(y1, t1, 0.5)
            return y1

        def newton_recip(dst_tag, a):
            r0 = small.tile([128, 1], f32, tag=dst_tag + "r0")
            nc.vector.reciprocal(r0, a)
            t1 = small.tile([128, 1], f32, tag=dst_tag + "t1")
            nc.vector.tensor_mul(t1, a, r0)
            nc.vector.tensor_scalar(out=t1, in0=t1, scalar1=-1.0, scalar2=2.0,
                                    op0=AL.mult, op1=AL.add)
            nc.vector.tensor_mul(r0, r0, t1)
            return r0

        raw = small.tile([128, 1], f32, tag="raw")
        nc.vector.tensor_scalar_mul(raw, gl[:, 3:4], 1.0 / (9.0 * N))
        lam2 = small.tile([128, 1], f32, tag="lam2")
        nc.vector.tensor_scalar(out=lam2, in0=raw, scalar1=1.0 / D,
                                scalar2=EPS, op0=AL.mult, op1=AL.add)
        lam = newton_sqrt("lam", lam2)
        rlam = newton_recip("rlam", lam)
        rlam2 = small.tile([128, 1], f32, tag="rlam2")
        nc.vector.tensor_mul(rlam2, rlam, rlam)
        drift = small.tile([128, 1], f32, tag="drift")
        nc.vector.tensor_mul(drift, raw, rlam2)
        loss = small.tile([128, 1], f32, tag="loss")
        nc.vector.tensor_scalar_mul(loss, drift, 1.0 / D)
        negf3 = small.tile([128, 1], f32, tag="negf3")
        nc.vector.tensor_scalar_mul(negf3, rlam, -1.0 / 3.0)

        st = small.tile([1, 8], f32, tag="st")
        nc.vector.memset(st, 0.0)
        nc.vector.tensor_copy(out=st[0:1, 0:1], in_=loss[0:1, :])
        nc.vector.tensor_copy(out=st[0:1, 1:2], in_=drift[0:1, :])
        nc.vector.tensor_copy(out=st[0:1, 2:3], in_=raw[0:1, :])
        nc.vector.tensor_copy(out=st[0:1, 3:4], in_=lam[0:1, :])
        for t in range(NT):
            nc.vector.tensor_scalar_mul(st[0:1, 4 + t:5 + t],
                                        gl[0:1, t:t + 1], 1.0 / N)
        nc.sync.dma_start(out=st_d, in_=st)

        # --- V out ---
        for it in range(N_IT):
            vo = outp.tile([128, D], f32, tag="vo")
            nc.vector.tensor_scalar_mul(vo, W[it], negf3)
            nc.sync.dma_start(out=v_d[it * 128:(it + 1) * 128, :], in_=vo)

    with tile.TileContext(nc) as tc, ExitStack() as ctx:
        _trace(tc, ctx)
    nc.compile()
    return nc


def _get_nc():
    if "nc" not in _CACHE:
        _install_ntff_hook()
        _CACHE["nc"] = build()
    return _CACHE["nc"]


def run(inputs, trace=False):
    from concourse import bass_utils

    nc = _get_nc()
    x = np.ascontiguousarray(inputs["x"], dtype=np.float32)
    yp = np.ascontiguousarray(inputs["y_pos"], dtype=np.float32)
    yn = np.ascontiguousarray(inputs["y_neg"], dtype=np.float32)
    in_maps = [
        {"x": x[c * I_LOC:(c + 1) * I_LOC], "y_pos": yp, "y_neg": yn}
        for c in range(N_CORES)
    ]
    res = bass_utils.run_bass_kernel_spmd(
        nc, in_maps, core_ids=list(range(N_CORES)), trace=trace)
    V = np.concatenate([res.results[c]["V"] for c in range(N_CORES)], axis=0)
    st = res.results[0]["stats"][0]
    out = (
        np.float32(st[0]),            # loss
        np.float32(st[1]),            # drift_norm
        np.float32(st[2]),            # raw_drift_norm
        np.float32(st[3]),            # lambda_V
        st[4:7].astype(np.float32),   # per_temp
        V,                            # V [4096, 256]
    )
    return out, res


def kernel(**inputs):
    out, _ = run(inputs, trace=False)
    return out
